# revision 1
# baseline (speedup 1.0000x reference)
"""Trainium2 Bass kernel for nn_AttentionSE3 (graph attention message passing).

Strategy (edge/graph parallel, fully host-prepped ELL layout):
- Attention is a segment softmax over incoming edges of each dst node.  Logits are
  dot(k_edge, q_dst)/sqrt(128) with k,q ~ N(0,1): |logit| <~ 2, so the max-subtraction
  is dropped (softmax is shift-invariant; exp() never overflows here) and
  out[n] = sum_e exp(logit_e) * v_e / sum_e exp(logit_e).
- Host sorts nodes by in-degree, packs them into 128-node blocks, and pads each
  block's per-node edge lists to the block max degree D (degree sorting makes the
  padding ~2%).  Blocks are dealt round-robin to the 8 cores; the per-group capacity
  is the max over the 8 cores so EVERY core runs the same static program (no
  collectives: no node's edges ever span two cores).
- Per (node, d) "slot" the host gathers the edge's key row [128] and value row [96]
  (zero for padding).  A padded slot contributes exactly exp(0)=1 to the softmax
  denominator, so the device subtracts a per-node pad count (exact correction).
  Zero-degree nodes get pad_count = D-1 so the denominator is exactly 1 and the
  output row is 0, matching segment_sum semantics.
- Device program per block: DMA k/v tiles [128 nodes x D*feat]; VectorE multiplies
  k by q (q broadcast over d), reduces dk->logits; ScalarE applies exp (with the
  1/sqrt(128) folded into the activation scale); VectorE reduces d->denominator,
  subtracts pad counts, reciprocates, weights v by exp(logits), reduces d, and
  normalizes.  A tunable share of the two big elementwise multiplies goes to GPSIMD
  to balance engines.  Output accumulates in SBUF and is stored with one DMA.
"""

import numpy as np

import concourse.bacc as bacc
import concourse.mybir as mybir
from concourse import tile
from concourse.bass_utils import run_bass_kernel_spmd

try:
    import ml_dtypes
    BF16_NP = np.dtype(ml_dtypes.bfloat16)
except ImportError:  # pragma: no cover
    BF16_NP = None

N_NODES = 50000
H = 8
P = 128  # nodes per block
N_CORES = 8
SCALE = float(1.0 / np.sqrt(128.0))
F32 = mybir.dt.float32

# Fraction of the d-range of the two big elementwise multiplies routed to GPSIMD
# (engine balancing; VectorE carries the reduces which it alone can do).
GP_FRAC_W1 = 0.70
GP_FRAC_W6 = 0.70

# "f32" or "bf16": dtype of k/v/q inputs and of the two weighting products
# (halves DMA traffic and doubles VectorE elementwise throughput; softmax
# accumulations stay fp32).
DTYPE_MODE = "bf16"
# Replace the dk-reduction (radix-16) with 4 pairwise-halves adds in bf16
# (bf16 tensor_tensor runs 2x; tensor_reduce is stuck at 1x).
TREE_W2 = True
# One pairwise-halves add over d before the weighted-value reduction (rounds
# block capacities up to even, ~+3% traffic; halves the strided 1x reduce).
TREE_W7 = True
# 2 = second halving level (capacities rounded to multiples of 4, ~+8% traffic;
# quarters the strided reduce).
TREE_W7_LEVELS = 1

# value columns permuted from [h(8), cx(12)] to [cx(12), h(8)] so the expw
# broadcast in the weighting multiply lands on a middle AP dim (stride-0 inner
# dims are ~6x slower on VectorE); output is produced in the same [cx, h]
# layout and un-permuted on the host.
PERM_V = np.arange(96).reshape(8, 12).T.reshape(-1)  # new_col cx*8+h -> old h*12+cx
PERM_V_INV = np.argsort(PERM_V)


# ---------------------------------------------------------------- host prep

def prepare(value, key, query0, query1, edge_index, n_nodes=N_NODES, n_cores=N_CORES):
    """Build per-core padded ELL shards.  Returns (in_maps, meta)."""
    value = np.asarray(value, dtype=np.float32)
    key = np.asarray(key, dtype=np.float32)
    query0 = np.asarray(query0, dtype=np.float32)
    query1 = np.asarray(query1, dtype=np.float32)
    n_edges = key.shape[0]

    dst = np.asarray(edge_index[1], dtype=np.int64)
    deg = np.bincount(dst, minlength=n_nodes).astype(np.int64)
    n_pad = -(-n_nodes // (P * n_cores)) * (P * n_cores)  # round up to 1024
    deg_pad = np.concatenate([deg, np.zeros(n_pad - n_nodes, dtype=np.int64)])
    nb = n_pad // P
    ng = nb // n_cores

    order = np.argsort(deg_pad, kind="stable")  # node ids, degree-ascending
    degs_o = deg_pad[order]

    blk_max = degs_o.reshape(nb, P).max(axis=1)
    D_eff = np.maximum(blk_max.reshape(ng, n_cores).max(axis=1), 1).astype(np.int64)
    if TREE_W7:
        m = 4 if TREE_W7_LEVELS >= 2 else 2
        D_eff = (D_eff + m - 1) // m * m  # capacities divisible for halving
    off = np.concatenate([[0], np.cumsum(P * D_eff)]).astype(np.int64)
    S = int(off[-1])  # slots per core

    pos = np.arange(n_pad)
    block = pos // P
    g_of = block // n_cores
    core_of = block % n_cores
    row = pos % P
    Dg = D_eff[g_of]
    base = off[g_of] + row * Dg

    edge_order = np.argsort(dst, kind="stable")
    starts = np.concatenate([[0], np.cumsum(deg)])

    pp = np.repeat(pos, degs_o)
    cum0 = np.concatenate([[0], np.cumsum(degs_o)])[:-1]
    d_idx = np.arange(n_edges) - np.repeat(cum0, degs_o)
    node_of_pp = order[pp]
    edge_ids = edge_order[starts[node_of_pp] + d_idx]
    slot_global = core_of[pp] * S + base[pp] + d_idx

    kp = np.zeros((n_cores * S, 128), dtype=np.float32)
    kp[slot_global] = key[edge_ids]
    vp = np.zeros((n_cores * S, 96), dtype=np.float32)
    vp[slot_global] = value.reshape(n_edges, 96)[:, PERM_V][edge_ids]
    kp = kp.reshape(n_cores, S, 128)
    vp = vp.reshape(n_cores, S, 96)

    qfull = np.concatenate([query0, query1], axis=-1).reshape(n_nodes, 128)
    q_pad = np.zeros((n_pad, 128), dtype=np.float32)
    q_pad[:n_nodes] = qfull
    q_sorted = q_pad[order].reshape(nb, P, 128)

    pc = (Dg - degs_o).astype(np.float32)
    zero_deg = degs_o == 0
    pc[zero_deg] = (Dg[zero_deg] - 1).astype(np.float32)
    pc_sorted = pc.reshape(nb, P)

    dt = BF16_NP if DTYPE_MODE == "bf16" else np.float32
    if DTYPE_MODE == "bf16":
        kp = kp.astype(dt)
        vp = vp.astype(dt)
    in_maps = []
    for c in range(n_cores):
        # pre-tiled layouts: q [128, ng*128], pc [128, ng]
        q_c = np.ascontiguousarray(
            q_sorted[c::n_cores].transpose(1, 0, 2).reshape(P, ng * 128)).astype(dt)
        # pad counts pre-expanded over heads -> the denominator subtract is a
        # plain contiguous tensor_tensor (scalar-AP operands load serially,
        # ~1.6us per op; broadcast APs are worse)
        pc_c = np.repeat(np.ascontiguousarray(pc_sorted[c::n_cores].T), H, axis=1)
        in_maps.append({"kp": kp[c], "vp": vp[c], "q": q_c, "pc": pc_c})

    meta = dict(D_eff=D_eff, off=off, S=S, NG=ng, NB=nb, order=order,
                n_nodes=n_nodes, n_pad=n_pad)
    return in_maps, meta


def unshard_output(out_cores, meta):
    """out_cores: list of [128, NG*96] -> [n_nodes, 32, 3]."""
    ng, nb = meta["NG"], meta["NB"]
    n_cores = len(out_cores)
    order, n_nodes, n_pad = meta["order"], meta["n_nodes"], meta["n_pad"]
    out_sorted = np.zeros((nb, P, 96), dtype=np.float32)
    for c in range(n_cores):
        out_sorted[c::n_cores] = (
            out_cores[c].reshape(P, ng, 96).transpose(1, 0, 2))
    out_sorted = out_sorted.reshape(n_pad, 96)[:, PERM_V_INV]
    out_full = np.zeros((n_nodes, 96), dtype=np.float32)
    mask = order < n_nodes
    out_full[order[mask]] = out_sorted[mask]
    return out_full.reshape(n_nodes, 32, 3)


# ---------------------------------------------------------------- bass kernel

def build(D_eff, S, NG, n_cores=N_CORES):
    D_eff = [int(d) for d in D_eff]
    off = np.concatenate([[0], np.cumsum([P * d for d in D_eff])]).astype(np.int64)

    nc = bacc.Bacc("TRN2", target_bir_lowering=False, debug=False,
                   num_devices=n_cores)
    DT = mybir.dt.bfloat16 if DTYPE_MODE == "bf16" else F32
    kp = nc.declare_dram_parameter("kp", [S, 128], DT, isOutput=False)
    vp = nc.declare_dram_parameter("vp", [S, 96], DT, isOutput=False)
    q = nc.declare_dram_parameter("q", [P, NG * 128], DT, isOutput=False)
    pc = nc.declare_dram_parameter("pc", [P, NG * H], F32, isOutput=False)
    out = nc.declare_dram_parameter("out", [P, NG * 96], F32, isOutput=True)

    mult = mybir.AluOpType.mult
    add = mybir.AluOpType.add
    AX = mybir.AxisListType.X

    with tile.TileContext(nc) as tc:
        with tc.tile_pool(name="res", bufs=1) as res, \
             tc.tile_pool(name="work", bufs=2) as work, \
             tc.tile_pool(name="small", bufs=3) as small:
            q_sb = res.tile([P, NG * 128], DT)
            nc.sync.dma_start(q_sb[:], q[:])
            pc_sb = res.tile([P, NG * H], F32)
            nc.sync.dma_start(pc_sb[:], pc[:])
            out_sb = res.tile([P, NG * 96], F32)
            ss_all = res.tile([P, NG * H], F32)

            for g in range(NG):
                D = D_eff[g]
                s0 = int(off[g])
                kt = work.tile([P, D * 128], DT, tag="kt")
                nc.sync.dma_start(
                    kt[:], kp[s0:s0 + P * D, :].rearrange("(n d) f -> n (d f)", n=P))
                vt = work.tile([P, D * 96], DT, tag="vt")
                nc.sync.dma_start(
                    vt[:], vp[s0:s0 + P * D, :].rearrange("(n d) f -> n (d f)", n=P))

                # w = k * q  (q broadcast over d)   [P, D, H, 16]
                qb = (q_sb[:, g * 128:(g + 1) * 128]
                      .rearrange("n (h k) -> n h k", h=H)
                      .unsqueeze(1).broadcast_to([P, D, H, 16]))
                w = work.tile([P, D * 128], DT, tag="kt")
                w4 = w[:].rearrange("n (d h k) -> n d h k", d=D, h=H)
                k4 = kt[:].rearrange("n (d h k) -> n d h k", d=D, h=H)
                dv = D - int(round(D * GP_FRAC_W1))
                if dv > 0:
                    nc.vector.tensor_tensor(
                        out=w4[:, :dv], in0=k4[:, :dv], in1=qb[:, :dv], op=mult)
                if dv < D:
                    nc.gpsimd.tensor_tensor(
                        out=w4[:, dv:], in0=k4[:, dv:], in1=qb[:, dv:], op=mult)

                # logits (unscaled) = reduce_k w   [P, D*H]
                lg = small.tile([P, D * H], F32, tag="lg")
                if TREE_W2:
                    # radix-16 sum as pairwise halves: bf16 TT runs 2x, reduce 1x
                    t8 = small.tile([P, D * H * 8], DT, tag="t8")
                    nc.vector.tensor_tensor(
                        out=t8[:].rearrange("n (a k) -> n a k", k=8),
                        in0=w[:].rearrange("n (a k) -> n a k", k=16)[:, :, :8],
                        in1=w[:].rearrange("n (a k) -> n a k", k=16)[:, :, 8:],
                        op=add)
                    t4 = small.tile([P, D * H * 4], DT, tag="t4")
                    nc.vector.tensor_tensor(
                        out=t4[:].rearrange("n (a k) -> n a k", k=4),
                        in0=t8[:].rearrange("n (a k) -> n a k", k=8)[:, :, :4],
                        in1=t8[:].rearrange("n (a k) -> n a k", k=8)[:, :, 4:],
                        op=add)
                    t2 = small.tile([P, D * H * 2], DT, tag="t2")
                    nc.vector.tensor_tensor(
                        out=t2[:].rearrange("n (a k) -> n a k", k=2),
                        in0=t4[:].rearrange("n (a k) -> n a k", k=4)[:, :, :2],
                        in1=t4[:].rearrange("n (a k) -> n a k", k=4)[:, :, 2:],
                        op=add)
                    nc.vector.tensor_tensor(
                        out=lg[:],
                        in0=t2[:].rearrange("n (a k) -> n a k", k=2)[:, :, 0],
                        in1=t2[:].rearrange("n (a k) -> n a k", k=2)[:, :, 1],
                        op=add)
                else:
                    nc.vector.tensor_reduce(
                        out=lg[:], in_=w[:].rearrange("n (dh k) -> n dh k", k=16),
                        axis=AX, op=add)

                # expw = exp(scale * logits)
                ew = small.tile([P, D * H], DT, tag="ew")
                nc.scalar.activation(out=ew[:], in_=lg[:],
                                     func=mybir.ActivationFunctionType.Exp,
                                     scale=SCALE)

                # segment sum straight into the resident tile; normalization is
                # deferred to one wide pass after the loop (keeps the tiny
                # subtract/reciprocal off every block's critical chain)
                nc.vector.tensor_reduce(
                    out=ss_all[:, g * H:(g + 1) * H],
                    in_=ew[:].rearrange("n (d h) -> n h d", d=D),
                    axis=AX, op=add)

                # wv = v * expw; v columns are [cx, h] so the expw broadcast is
                # on the middle dim and the inner stays contiguous
                wv = work.tile([P, D * 96], DT, tag="vt")
                wv4 = wv[:].rearrange("n (d c h) -> n d c h", d=D, c=12)
                v4 = vt[:].rearrange("n (d c h) -> n d c h", d=D, c=12)
                eb = (ew[:].rearrange("n (d h) -> n d h", d=D)
                      .unsqueeze(2).broadcast_to([P, D, 12, H]))
                dv6 = D - int(round(D * GP_FRAC_W6))
                if dv6 > 0:
                    nc.vector.tensor_tensor(
                        out=wv4[:, :dv6], in0=v4[:, :dv6], in1=eb[:, :dv6], op=mult)
                if dv6 < D:
                    nc.gpsimd.tensor_tensor(
                        out=wv4[:, dv6:], in0=v4[:, dv6:], in1=eb[:, dv6:], op=mult)

                # unnormalized out = reduce_d wv, straight into out_sb
                og = out_sb[:, g * 96:(g + 1) * 96]
                if TREE_W7:
                    Dh = D // 2
                    th = small.tile([P, Dh * 96], DT, tag="th")
                    wv3 = wv[:].rearrange("n (d ch) -> n d ch", d=D)
                    nc.vector.tensor_tensor(
                        out=th[:].rearrange("n (d ch) -> n d ch", d=Dh),
                        in0=wv3[:, :Dh], in1=wv3[:, Dh:], op=add)
                    red, rd = th, Dh
                    if TREE_W7_LEVELS >= 2:
                        Dq = Dh // 2
                        tq = small.tile([P, Dq * 96], DT, tag="tq")
                        th3 = th[:].rearrange("n (d ch) -> n d ch", d=Dh)
                        nc.vector.tensor_tensor(
                            out=tq[:].rearrange("n (d ch) -> n d ch", d=Dq),
                            in0=th3[:, :Dq], in1=th3[:, Dq:], op=add)
                        red, rd = tq, Dq
                    nc.vector.tensor_reduce(
                        out=og, in_=red[:].rearrange("n (d ch) -> n ch d", d=rd),
                        axis=AX, op=add)
                else:
                    nc.vector.tensor_reduce(
                        out=og, in_=wv[:].rearrange("n (d ch) -> n ch d", d=D),
                        axis=AX, op=add)

            # one wide deferred normalization pass
            dn_all = res.tile([P, NG * H], F32)
            nc.vector.tensor_sub(out=dn_all[:], in0=ss_all[:], in1=pc_sb[:])
            rs_all = res.tile([P, NG * H], F32)
            nc.vector.reciprocal(out=rs_all[:], in_=dn_all[:])
            out2 = res.tile([P, NG * 96], F32)
            nc.vector.tensor_tensor(
                out=out2[:].rearrange("n (g c h) -> n g c h", g=NG, c=12),
                in0=out_sb[:].rearrange("n (g c h) -> n g c h", g=NG, c=12),
                in1=(rs_all[:].rearrange("n (g h) -> n g h", g=NG)
                     .unsqueeze(2).broadcast_to([P, NG, 12, H])),
                op=mult)

            nc.sync.dma_start(out[:], out2[:])

    nc.compile()
    return nc


# ---------------------------------------------------------------- entry point

LAST_RESULT = None  # BassKernelResults of the most recent run (for test harness)


def kernel(value, key, query0, query1, edge_index):
    global LAST_RESULT
    import os
    in_maps, meta = prepare(value, key, query0, query1, edge_index)
    nc = build(meta["D_eff"], meta["S"], meta["NG"])
    res = run_bass_kernel_spmd(nc, in_maps, list(range(N_CORES)),
                               tmpdir=os.environ.get("BASS_SPMD_TMPDIR"))
    LAST_RESULT = res
    out_cores = [res.results[c]["out"] for c in range(N_CORES)]
    return unshard_output(out_cores, meta)



# revision 2
# speedup vs baseline: 1.3516x; 1.3516x over previous
"""Trainium2 Bass kernel for nn_AttentionSE3 (graph attention message passing).

Strategy (edge/graph parallel, fully host-prepped ELL layout):
- Attention is a segment softmax over incoming edges of each dst node.  Logits are
  dot(k_edge, q_dst)/sqrt(128) with k,q ~ N(0,1): |logit| <~ 2, so the max-subtraction
  is dropped (softmax is shift-invariant; exp() never overflows here) and
  out[n] = sum_e exp(logit_e) * v_e / sum_e exp(logit_e).
- Host sorts nodes by in-degree, packs them into 128-node blocks, and pads each
  block's per-node edge lists to the block max degree D (degree sorting makes the
  padding ~2%).  Blocks are dealt round-robin to the 8 cores; the per-group capacity
  is the max over the 8 cores so EVERY core runs the same static program (no
  collectives: no node's edges ever span two cores).
- Per (node, d) "slot" the host gathers the edge's key row [128] and value row [96]
  (zero for padding).  A padded slot contributes exactly exp(0)=1 to the softmax
  denominator, so the device subtracts a per-node pad count (exact correction).
  Zero-degree nodes get pad_count = D-1 so the denominator is exactly 1 and the
  output row is 0, matching segment_sum semantics.
- ALL compute stays on VectorE + ScalarE.  GPSIMD shares an SBUF port with
  VectorE; measured on HW, a DVE tensor_tensor slows down 7-9x while any GPSIMD
  tensor op runs, so offloading elementwise work to GPSIMD is a large net loss.
- Device program per block: DMA k/v tiles [128 nodes x D*feat]; VectorE multiplies
  k by q (q broadcast over d, bf16 2x mode), one radix-8 pairwise add then a
  contiguous-inner tensor_reduce gives logits; ScalarE applies exp (1/sqrt(128)
  folded into the activation scale); VectorE reduces d->denominator, weights v by
  expw (broadcast on the middle AP dim, 2x mode), one pairwise halving then a
  reduce over d gives the unnormalized output.  Normalization is one deferred
  wide pass at the end.  Output accumulates in SBUF, stored with one DMA.
"""

import numpy as np

import concourse.bacc as bacc
import concourse.mybir as mybir
from concourse import tile
from concourse.bass_utils import run_bass_kernel_spmd

try:
    import ml_dtypes
    BF16_NP = np.dtype(ml_dtypes.bfloat16)
except ImportError:  # pragma: no cover
    BF16_NP = None

N_NODES = 50000
H = 8
P = 128  # nodes per block
N_CORES = 8
SCALE = float(1.0 / np.sqrt(128.0))
F32 = mybir.dt.float32

# Logit reduction: "tree8" = one pairwise radix-8 add (bf16 2x) then a contiguous
# tensor_reduce over 8; "direct" = single tensor_reduce over 16.
LOGIT_MODE = "tree8"
# One pairwise-halves add over d before the weighted-value reduction (rounds
# block capacities up to even, ~+3% traffic; halves the strided 1x reduce).
TREE_W7 = True

# value columns permuted from [h(8), cx(12)] to [cx(12), h(8)] so the expw
# broadcast in the weighting multiply lands on a middle AP dim; output is
# produced in the same [cx, h] layout and un-permuted on the host.
PERM_V = np.arange(96).reshape(8, 12).T.reshape(-1)  # new_col cx*8+h -> old h*12+cx
PERM_V_INV = np.argsort(PERM_V)


# ---------------------------------------------------------------- host prep

def prepare(value, key, query0, query1, edge_index, n_nodes=N_NODES, n_cores=N_CORES):
    """Build per-core padded ELL shards.  Returns (in_maps, meta)."""
    value = np.asarray(value, dtype=np.float32)
    key = np.asarray(key, dtype=np.float32)
    query0 = np.asarray(query0, dtype=np.float32)
    query1 = np.asarray(query1, dtype=np.float32)
    n_edges = key.shape[0]

    dst = np.asarray(edge_index[1], dtype=np.int64)
    deg = np.bincount(dst, minlength=n_nodes).astype(np.int64)
    n_pad = -(-n_nodes // (P * n_cores)) * (P * n_cores)  # round up to 1024
    deg_pad = np.concatenate([deg, np.zeros(n_pad - n_nodes, dtype=np.int64)])
    nb = n_pad // P
    ng = nb // n_cores

    order = np.argsort(deg_pad, kind="stable")  # node ids, degree-ascending
    degs_o = deg_pad[order]

    blk_max = degs_o.reshape(nb, P).max(axis=1)
    D_eff = np.maximum(blk_max.reshape(ng, n_cores).max(axis=1), 1).astype(np.int64)
    if TREE_W7:
        D_eff = (D_eff + 1) // 2 * 2  # capacities even for the halving add
    off = np.concatenate([[0], np.cumsum(P * D_eff)]).astype(np.int64)
    S = int(off[-1])  # slots per core

    pos = np.arange(n_pad)
    block = pos // P
    g_of = block // n_cores
    core_of = block % n_cores
    row = pos % P
    Dg = D_eff[g_of]
    base = off[g_of] + row * Dg

    edge_order = np.argsort(dst, kind="stable")
    starts = np.concatenate([[0], np.cumsum(deg)])

    pp = np.repeat(pos, degs_o)
    cum0 = np.concatenate([[0], np.cumsum(degs_o)])[:-1]
    d_idx = np.arange(n_edges) - np.repeat(cum0, degs_o)
    node_of_pp = order[pp]
    edge_ids = edge_order[starts[node_of_pp] + d_idx]
    slot_global = core_of[pp] * S + base[pp] + d_idx

    kp = np.zeros((n_cores * S, 128), dtype=np.float32)
    kp[slot_global] = key[edge_ids]
    vp = np.zeros((n_cores * S, 96), dtype=np.float32)
    vp[slot_global] = value.reshape(n_edges, 96)[:, PERM_V][edge_ids]
    kp = kp.reshape(n_cores, S, 128)
    vp = vp.reshape(n_cores, S, 96)

    qfull = np.concatenate([query0, query1], axis=-1).reshape(n_nodes, 128)
    q_pad = np.zeros((n_pad, 128), dtype=np.float32)
    q_pad[:n_nodes] = qfull
    q_sorted = q_pad[order].reshape(nb, P, 128)

    pc = (Dg - degs_o).astype(np.float32)
    zero_deg = degs_o == 0
    pc[zero_deg] = (Dg[zero_deg] - 1).astype(np.float32)
    pc_sorted = pc.reshape(nb, P)

    dt = BF16_NP
    kp = kp.astype(dt)
    vp = vp.astype(dt)
    in_maps = []
    for c in range(n_cores):
        # pre-tiled layouts: q [128, ng*128], pc [128, ng]
        q_c = np.ascontiguousarray(
            q_sorted[c::n_cores].transpose(1, 0, 2).reshape(P, ng * 128)).astype(dt)
        # pad counts pre-expanded over heads -> the denominator subtract is a
        # plain contiguous tensor_tensor
        pc_c = np.repeat(np.ascontiguousarray(pc_sorted[c::n_cores].T), H, axis=1)
        in_maps.append({"kp": kp[c], "vp": vp[c], "q": q_c, "pc": pc_c})

    meta = dict(D_eff=D_eff, off=off, S=S, NG=ng, NB=nb, order=order,
                n_nodes=n_nodes, n_pad=n_pad)
    return in_maps, meta


def unshard_output(out_cores, meta):
    """out_cores: list of [128, NG*96] -> [n_nodes, 32, 3]."""
    ng, nb = meta["NG"], meta["NB"]
    n_cores = len(out_cores)
    order, n_nodes, n_pad = meta["order"], meta["n_nodes"], meta["n_pad"]
    out_sorted = np.zeros((nb, P, 96), dtype=np.float32)
    for c in range(n_cores):
        out_sorted[c::n_cores] = (
            out_cores[c].reshape(P, ng, 96).transpose(1, 0, 2))
    out_sorted = out_sorted.reshape(n_pad, 96)[:, PERM_V_INV]
    out_full = np.zeros((n_nodes, 96), dtype=np.float32)
    mask = order < n_nodes
    out_full[order[mask]] = out_sorted[mask]
    return out_full.reshape(n_nodes, 32, 3)


# ---------------------------------------------------------------- bass kernel

def build(D_eff, S, NG, n_cores=N_CORES):
    D_eff = [int(d) for d in D_eff]
    off = np.concatenate([[0], np.cumsum([P * d for d in D_eff])]).astype(np.int64)

    nc = bacc.Bacc("TRN2", target_bir_lowering=False, debug=False,
                   num_devices=n_cores)
    DT = mybir.dt.bfloat16
    kp = nc.declare_dram_parameter("kp", [S, 128], DT, isOutput=False)
    vp = nc.declare_dram_parameter("vp", [S, 96], DT, isOutput=False)
    q = nc.declare_dram_parameter("q", [P, NG * 128], DT, isOutput=False)
    pc = nc.declare_dram_parameter("pc", [P, NG * H], F32, isOutput=False)
    out = nc.declare_dram_parameter("out", [P, NG * 96], F32, isOutput=True)

    mult = mybir.AluOpType.mult
    add = mybir.AluOpType.add
    AX = mybir.AxisListType.X

    with tile.TileContext(nc) as tc:
        with tc.tile_pool(name="res", bufs=1) as res, \
             tc.tile_pool(name="work", bufs=2) as work, \
             tc.tile_pool(name="small", bufs=3) as small:
            q_sb = res.tile([P, NG * 128], DT)
            nc.sync.dma_start(q_sb[:], q[:])
            pc_sb = res.tile([P, NG * H], F32)
            nc.sync.dma_start(pc_sb[:], pc[:])
            out_sb = res.tile([P, NG * 96], F32)
            ss_all = res.tile([P, NG * H], F32)

            for g in range(NG):
                D = D_eff[g]
                s0 = int(off[g])
                kt = work.tile([P, D * 128], DT, tag="kt")
                nc.sync.dma_start(
                    kt[:], kp[s0:s0 + P * D, :].rearrange("(n d) f -> n (d f)", n=P))
                vt = work.tile([P, D * 96], DT, tag="vt")
                nc.sync.dma_start(
                    vt[:], vp[s0:s0 + P * D, :].rearrange("(n d) f -> n (d f)", n=P))

                # w = k * q  (q broadcast over d: outer stride-0, bf16 2x mode)
                qb = (q_sb[:, g * 128:(g + 1) * 128]
                      .rearrange("n (h k) -> n h k", h=H)
                      .unsqueeze(1).broadcast_to([P, D, H, 16]))
                w = work.tile([P, D * 128], DT, tag="kt")
                nc.vector.tensor_tensor(
                    out=w[:].rearrange("n (d h k) -> n d h k", d=D, h=H),
                    in0=kt[:].rearrange("n (d h k) -> n d h k", d=D, h=H),
                    in1=qb, op=mult)

                # logits (unscaled) = reduce_k w   [P, D*H] f32
                lg = small.tile([P, D * H], F32, tag="lg")
                if LOGIT_MODE == "tree8":
                    # radix-8 pairwise add (bf16 2x), then contiguous reduce over 8
                    t8 = small.tile([P, D * H * 8], DT, tag="t8")
                    nc.vector.tensor_tensor(
                        out=t8[:].rearrange("n (a k) -> n a k", k=8),
                        in0=w[:].rearrange("n (a k) -> n a k", k=16)[:, :, :8],
                        in1=w[:].rearrange("n (a k) -> n a k", k=16)[:, :, 8:],
                        op=add)
                    nc.vector.tensor_reduce(
                        out=lg[:], in_=t8[:].rearrange("n (a k) -> n a k", k=8),
                        axis=AX, op=add)
                else:
                    nc.vector.tensor_reduce(
                        out=lg[:], in_=w[:].rearrange("n (dh k) -> n dh k", k=16),
                        axis=AX, op=add)

                # expw = exp(scale * logits)  (ScalarE; contiguous write)
                ew = small.tile([P, D * H], DT, tag="ew")
                nc.scalar.activation(out=ew[:], in_=lg[:],
                                     func=mybir.ActivationFunctionType.Exp,
                                     scale=SCALE)

                # segment sum straight into the resident tile; normalization is
                # deferred to one wide pass after the loop
                nc.vector.tensor_reduce(
                    out=ss_all[:, g * H:(g + 1) * H],
                    in_=ew[:].rearrange("n (d h) -> n h d", d=D),
                    axis=AX, op=add)

                # wv = v * expw; v columns are [cx, h] so the expw broadcast is
                # on the middle dim and the inner stays contiguous (2x mode)
                wv = work.tile([P, D * 96], DT, tag="vt")
                eb = (ew[:].rearrange("n (d h) -> n d h", d=D)
                      .unsqueeze(2).broadcast_to([P, D, 12, H]))
                nc.vector.tensor_tensor(
                    out=wv[:].rearrange("n (d c h) -> n d c h", d=D, c=12),
                    in0=vt[:].rearrange("n (d c h) -> n d c h", d=D, c=12),
                    in1=eb, op=mult)

                # unnormalized out = reduce_d wv, straight into out_sb
                og = out_sb[:, g * 96:(g + 1) * 96]
                if TREE_W7:
                    Dh = D // 2
                    th = small.tile([P, Dh * 96], DT, tag="th")
                    wv3 = wv[:].rearrange("n (d ch) -> n d ch", d=D)
                    nc.vector.tensor_tensor(
                        out=th[:].rearrange("n (d ch) -> n d ch", d=Dh),
                        in0=wv3[:, :Dh], in1=wv3[:, Dh:], op=add)
                    nc.vector.tensor_reduce(
                        out=og, in_=th[:].rearrange("n (d ch) -> n ch d", d=Dh),
                        axis=AX, op=add)
                else:
                    nc.vector.tensor_reduce(
                        out=og, in_=wv[:].rearrange("n (d ch) -> n ch d", d=D),
                        axis=AX, op=add)

            # one wide deferred normalization pass
            dn_all = res.tile([P, NG * H], F32)
            nc.vector.tensor_sub(out=dn_all[:], in0=ss_all[:], in1=pc_sb[:])
            rs_all = res.tile([P, NG * H], F32)
            nc.vector.reciprocal(out=rs_all[:], in_=dn_all[:])
            out2 = res.tile([P, NG * 96], F32)
            nc.vector.tensor_tensor(
                out=out2[:].rearrange("n (g c h) -> n g c h", g=NG, c=12),
                in0=out_sb[:].rearrange("n (g c h) -> n g c h", g=NG, c=12),
                in1=(rs_all[:].rearrange("n (g h) -> n g h", g=NG)
                     .unsqueeze(2).broadcast_to([P, NG, 12, H])),
                op=mult)

            nc.sync.dma_start(out[:], out2[:])

    nc.compile()
    return nc


# ---------------------------------------------------------------- entry point

LAST_RESULT = None  # BassKernelResults of the most recent run (for test harness)


def kernel(value, key, query0, query1, edge_index):
    global LAST_RESULT
    import os
    in_maps, meta = prepare(value, key, query0, query1, edge_index)
    nc = build(meta["D_eff"], meta["S"], meta["NG"])
    res = run_bass_kernel_spmd(nc, in_maps, list(range(N_CORES)),
                               tmpdir=os.environ.get("BASS_SPMD_TMPDIR"))
    LAST_RESULT = res
    out_cores = [res.results[c]["out"] for c in range(N_CORES)]
    return unshard_output(out_cores, meta)
